# revision 87
# baseline (speedup 1.0000x reference)
"""RNN-T joiner (nn_CombinationModel_53154515256115) as a Bass/Tile SPMD kernel
for 8 Trainium2 NeuronCores.

Algorithm
---------
For each valid (b, t, u):
    out[b,t,u] = relu(enc[b,t] @ Wj1_enc + pred[b,u] @ Wj1_pred + bj1) @ Wj2 + bj2
The joint pre-activation factors into a per-(b,t) term A and a per-(b,u) term
Pp. The dominant [N,640] @ [640,1056] output matmul runs on the PE in fp8-e4m3
DoubleRow mode (2 fp8 weights per cell, 256-wide contraction per instruction)
with a two-sided residual correction to keep precision:

    h ~= h8 + r8          (h8 = e4m3(h), r8 = e4m3(h - h8))
    W ~= W8 + R8          (host-side split of Wj2, scaled by 1/sw)
    out = h8@W8 + r8@W8 + h8@R8      (r8@R8 ~ 1e-3 relative, dropped)

All three block-products are packed into 7 DoubleRow k-pairs over a 10-slot
fp8 "h stack" (slots 0-4 = h8 k-tiles, 5-9 = r8 k-tiles); the W-side pair
tiles are prebuilt on the host so each pair picks the right (W8|R8) slices.
W k-tile 4 stays uncorrected (the 8th pair is dropped), and the r8 residual
for k-tile 4 is additionally skipped on the early small batches
(R4_DROP_BATCHES) to relieve the DVE/Pool expansion pipeline during the
fill phase; measured rel_rms 1.43e-2 vs the 2e-2 gate. Scales sh=1/16
(folded into Wj1 on the host) and sw=1/512 keep everything in e4m3 normal
range; the combined 2^-13 descale is folded into the PSUM evacuation.
Output is written bf16 and upcast on the host.

Schedule highlights (cost-model-driven; 195.8us -> 120.9us):
  - serial-DMA-device order: ball(p-major), eT, wp1, wp2, pk16, wj1p,
    wsrc W-slots (x2), wsrc R-slots (deferred; first read at pair q5)
  - pred-net/at PSUM tiles alternate ps_small/ps_main (4 effective buffers
    halve the PE<->ACT ping-pong latency); at_j and pp_j interleave per j
    so pp's evac chain (which gates the Pool expansion start) begins ASAP
  - expansion: Pool broadcast-add, DVE h8/hb/r8; all pieces emitted
    upfront (rings self-pace); pred-net u-axis packed to 296 valid slots
  - final tile runs chunk-major (per-512-col PSUM chunks evacuated on
    DVE/ACT and DMA'd while later chunks compute) to cut the drain tail

Sharding (SPMD-uniform)
-----------------------
Core c takes encoder frames t with t % 8 == c from every batch. Batches are
laid out smallest-first so the first output row-tiles become ready with the
least expansion work. The tiny prediction network (296 rows) is computed
replicated on every core.
"""

import numpy as np

import concourse.bass as bass
import concourse.mybir as mybir
import concourse.tile as tile
from concourse import bacc
from concourse.masks import make_identity
from concourse.bass import IndirectOffsetOnAxis
from concourse.bass_utils import run_bass_kernel_spmd

F32 = mybir.dt.float32
BF16 = mybir.dt.bfloat16
FP8 = mybir.dt.float8e4
I32 = mybir.dt.int32
AF = mybir.ActivationFunctionType

# ---------------------------------------------------------------- constants
B, T, U = 8, 300, 40
E, P, J, V = 512, 640, 640, 1056
H, DEMB = 2, 256
ENC_SIZES = [300, 280, 260, 240, 220, 210, 205, 200]
TGT_SIZES = [40, 38, 35, 33, 30, 28, 26, 25]
NCORES = 8
N_FLAT = 64385

G = [(t + NCORES - 1) // NCORES for t in ENC_SIZES]       # groups/core/batch
UB1 = [u + 1 for u in TGT_SIZES]                          # u-extent per batch
RBV = [G[b] * UB1[b] for b in range(B)]                   # valid rows/batch
ROWS = sum(RBV)                                           # 8134 rows/core
GT_TOT = sum(G)                                           # 242 enc frames/core
GT_PAD = 256
OFF_T = [0]
for b in range(B):
    OFF_T.append(OFF_T[-1] + G[b])

# batches laid out smallest-first in the row dimension
BATCH_ORDER = sorted(range(B), key=lambda b: RBV[b])
OFF_R = {}
_acc = 0
for b in BATCH_ORDER:
    OFF_R[b] = _acc
    _acc += RBV[b]

UOFF = [0]                    # packed (b,u) offsets: sum(U_b+1) = 296
for b in range(B):
    UOFF.append(UOFF[-1] + UB1[b])
NBU = UOFF[-1]                # 296 valid u-slots (vs 41*B = 328 padded)
NBU_PAD = 304                 # eT column pad (% 16)
KJ1_ENC = E // 128            # 4 k-tiles of W_j1 enc part
NJ = J // 128                 # 5 partition tiles of the 640-dim feature axis
V_CHUNKS = [(0, 512), (512, 512), (1024, V - 1024)]
MAXRV = max(RBV)

SH = 1.0 / 16.0               # h scale  (folded into Wj1/bj1 on host)
SP8 = 1.0 / 64.0              # pred-net fp8 weight scale (undone at activations)
SE8 = 1.0 / 64.0              # embedding fp8 scale
SW = 1.0 / 512.0              # W2 scale (folded into W8/R8 on host)
SEE8 = 1.0 / 16.0             # enc fp8 scale (at-loop DoubleRow)
SWE8 = 1.0 / 128.0            # wj1e fp8 scale (at-loop DoubleRow)
DESCALE = SH * SW             # 2^-13, exact

ROWS_PAD = 8192               # slot stride in the h-stack; % 16 == 0
NSLOT = 10                    # 5 h8 + 5 r8
# DoubleRow pair table: (h-slot pair base, W-source description)
# pairs q: h-side slots (2q mod 10, +1); W tiles prebuilt host-side:
#   q0 (h0,h1)x(W0,W1)  q1 (h2,h3)x(W2,W3)  q2 (h4,r0)x(W4,W0)
#   q3 (r1,r2)x(W1,W2)  q4 (r3,r4)x(W3,W4)
#   q5 (h0,h1)x(R0,R1)  q6 (h2,h3)x(R2,R3)  q7 (h4,r0)x(R4,0)
# NPAIRS=7 drops the (h8_4, r8_0) x (R8_4, 0) pair: leaves W k-tile 4
# uncorrected on the W side (~1.2e-2 predicted rel err vs 3.0e-3 for 8)
NPAIRS = 7
PAIR_HSLOT = [0, 2, 4, 6, 8, 0, 2, 4][:NPAIRS]
WSRC_OFF = (0, 2, 4, 1, 3, 6, 8, 10)[:NPAIRS]
NWSLOT = 10 if NPAIRS == 7 else 12

# columns [0, SPLIT6) also skip the q6 (h8 x R2,R3) correction pair: those
# columns carry W-quant error on 3/5 k-tiles (2.15e-2) instead of 1/5
# (1.24e-2); blended rel_rms ~= sqrt(f*4.62 + (1-f)*1.54)e-2 ~= 1.6e-2 at
# f = 1/3, saving SPLIT6/2 PE cycles per row-tile
SPLIT6 = 0
CH_LO = [(0, SPLIT6)] if SPLIT6 else []
CH_HI = ([(SPLIT6, 512 - SPLIT6)] if SPLIT6 else []) + \
    [(512, 512), (1024, V - 1024)] + ([] if SPLIT6 else [(0, 512)])
CH_HI.sort()
ALL_CH = sorted(CH_LO + CH_HI)

NTILES = (ROWS + 127) // 128
# batches whose h8-relu runs on ACT instead of DVE (engine balancing);
# empty: ACT must stay dedicated to PSUM evacuation or the PE stalls
H8_ACT_BATCHES = set()
# r8 (h-residual) is skipped for j=4 on the early batches: spends error
# budget (h-quant on 1/5 of K for ~37%% of rows, +~0.8e-2 rms in quadrature)
# to cut DVE expansion work exactly where the fill-phase lag stalls the PE.
# ACT can't help instead: its in-order queue delays PSUM evac (-> 144us).
R4_DROP_BATCHES = {7, 6, 5, 4}
FILL_AT = 0                  # PE fillers before the at-loop (pk16 wait)
FILL_MAIN = 0                 # PE fillers before main tile 0 (hs wait)

_cache = {}


def _build(bias2_zero=True):
    nc = bacc.Bacc("TRN2", target_bir_lowering=False, debug=False,
                   num_devices=NCORES)

    eT_d = nc.dram_tensor("eT", [2 * 128, 2 * NBU_PAD], FP8, kind="ExternalInput").ap()
    wp1_d = nc.dram_tensor("w_p1", [2 * 128, 2 * P], FP8, kind="ExternalInput").ap()
    wp2_d = nc.dram_tensor("w_p2", [3 * 128, 2 * P], FP8, kind="ExternalInput").ap()
    wj1p_d = nc.dram_tensor("w_j1p", [3 * 128, 2 * J], FP8, kind="ExternalInput").ap()
    # packed bf16: [encT | wj1e]
    PK16 = 4 * GT_PAD + 4 * J
    pk16_d = nc.dram_tensor("pk16", [128, PK16], BF16, kind="ExternalInput").ap()
    wsrc_d = nc.dram_tensor("w_src", [NWSLOT * 128, V], FP8,
                             kind="ExternalInput").ap()
    ball_d = nc.dram_tensor("b_all", [128, 15], F32, kind="ExternalInput").ap()
    bj2_d = nc.dram_tensor("b_j2", [V], F32, kind="ExternalInput").ap()
    out_d = nc.dram_tensor("out", [ROWS, V], BF16, kind="ExternalOutput").ap()

    from contextlib import ExitStack
    with tile.TileContext(nc) as tc, ExitStack() as ctx:
        persist = ctx.enter_context(tc.tile_pool(name="persist", bufs=1))
        stage = ctx.enter_context(tc.tile_pool(name="stage", bufs=1))
        expand = ctx.enter_context(tc.tile_pool(name="expand", bufs=5))
        hbpool = ctx.enter_context(tc.tile_pool(name="hbpool", bufs=3))
        outp = ctx.enter_context(tc.tile_pool(name="outp", bufs=3))
        ps_small = ctx.enter_context(tc.tile_pool(name="ps_small", bufs=2, space="PSUM"))
        ps_main = ctx.enter_context(tc.tile_pool(name="ps_main", bufs=2, space="PSUM"))

        # ---------------- PE warmup: build a >3us busy streak while DMAs run
        warm = stage.tile([128, 256], BF16, tag="warm", name="warm")
        warm_ps = [None]

        def emit_fillers(n):
            # dummy matmuls keep the PE busy-streak alive across known DMA
            # waits so the preamble runs at full p-state (idle >100ns drops
            # the clock to 1.2GHz for the next 3us of work)
            for _ in range(n):
                # main ring, not ps_small: keeps the pred-net's PSUM slots
                # free of WAW deps against the warm stream
                psw = ps_main.tile([128, V], F32, tag="ps_out",
                                   name="ps_warm")
                nc.tensor.matmul(out=psw[0:128, 0:256], lhsT=warm[:, 0:128],
                                 rhs=warm[:], start=True, stop=True)

        nc.vector.memset(warm[:], 0.0)
        # dummy gpsimd op hoists the Q7 library load into the idle window
        glib = stage.tile([128, 16], BF16, tag="glib", name="glib")
        nc.gpsimd.memset(glib[:], 0.0)
        emit_fillers(14)

        # ---------------- persistent SBUF state
        hs = persist.tile([128, NSLOT, ROWS_PAD], FP8, tag="hs", name="hs")
        drop_rows = sum(RBV[b] for b in R4_DROP_BATCHES)
        if drop_rows:
            # dropped r4 region must read as zero in pair q4 (DVE is idle
            # for ~9us at the start, so this costs no critical-path time)
            nc.vector.memset(hs[:, 9, 0:drop_rows], 0.0)
        at_t = [persist.tile([128, GT_TOT], F32, tag=f"at_{j}", name=f"at_{j}")
                for j in range(NJ)]
        pp_t = [persist.tile([128, NBU], F32, tag=f"pp_{j}", name=f"pp_{j}")
                for j in range(NJ)]

        # ---------------- weight loads: one strided DMA per tensor, ordered
        # by when the dependent compute needs them (DMA device is serial).
        # wsrc splits into the W-slots (needed at main-loop start) and the
        # R-slots (first read by pair q5, much later). ball ships [128,15]
        # p-major (contiguous 60B/partition descriptors) and goes first: the
        # biases gate every activation stage.
        ball_t = stage.tile([128, 15], F32, tag="ball")
        nc.sync.dma_start(out=ball_t[:], in_=ball_d)
        bp1_t = ball_t[:, 0:5]
        bp2_t = ball_t[:, 5:10]
        bj1_t = ball_t[:, 10:15]
        wp1p = stage.tile([128, 2, 2, P], FP8, tag="wp1", name="wp1")
        nc.sync.dma_start(out=wp1p[:].rearrange("p q a b -> p q (a b)"),
                          in_=wp1_d.rearrange("(q p) x -> p q x", p=128))
        eTs = stage.tile([128, 2, 2, NBU_PAD], FP8, tag="eT", name="eT")
        nc.sync.dma_start(out=eTs[:].rearrange("p q a b -> p q (a b)"),
                          in_=eT_d.rearrange("(q p) x -> p q x", p=128))
        wp2p = stage.tile([128, 3, 2, P], FP8, tag="wp2", name="wp2")
        nc.sync.dma_start(out=wp2p[:].rearrange("p q a b -> p q (a b)"),
                          in_=wp2_d.rearrange("(q p) x -> p q x", p=128))
        pk16 = stage.tile([128, PK16], BF16, tag="pk16", name="pk16")
        nc.sync.dma_start(out=pk16[:], in_=pk16_d)
        encT = [pk16[:, f * GT_PAD:(f + 1) * GT_PAD] for f in range(KJ1_ENC)]
        wj1e_b = [pk16[:, 4 * GT_PAD + k * J:4 * GT_PAD + (k + 1) * J]
                  for k in range(4)]
        wj1pp = stage.tile([128, 3, 2, J], FP8, tag="wj1p", name="wj1p")
        nc.sync.dma_start(out=wj1pp[:].rearrange("p q a b -> p q (a b)"),
                          in_=wj1p_d.rearrange("(q p) x -> p q x", p=128))
        if not bias2_zero:
            bt_t = persist.tile([128, V], BF16, tag="bt", name="bt")
            nc.sync.dma_start(out=bt_t[:], in_=bj2_d[None, :].to_broadcast([128, V]))
        # 12 W-slots [W0,W1,W2,W3,W4,W0, R0,R1,R2,R3,R4, 0]; each DoubleRow
        # pair reads two adjacent slots (overlapping views give the shifted
        # (W1,W2)/(W3,W4) pairs without duplicate storage)
        wsrc = persist.tile([128, NWSLOT, V], FP8, tag="wsrc", name="wsrc")
        nc.sync.dma_start(out=wsrc[:],
                          in_=wsrc_d.rearrange("(s p) v -> p s v", p=128))
        wp8 = [wsrc[:, o:o + 2] for o in WSRC_OFF]

        # ---------------- prediction network (replicated, 328 rows, fp8 pairs)
        # PSUM tiles alternate between ps_small and the main ring (idle until
        # tile 0) — 4 effective buffers halve the PE<->ACT ping-pong latency
        # that otherwise delays the expansion start by ~4us
        def ps_pre(j, n, name):
            if j % 2:
                return ps_main.tile([128, V], F32, tag="ps_out",
                                    name=name)[:, 0:n]
            return ps_small.tile([128, n], F32, tag="ps_s", name=name)

        h1s = stage.tile([128, 6, NBU_PAD], FP8, tag="h1s", name="h1s")
        pred_s = stage.tile([128, 6, NBU_PAD], FP8, tag="pred_s", name="pred_s")
        nc.vector.memset(h1s[:, 5], 0.0)
        nc.vector.memset(pred_s[:, 5], 0.0)
        NB = NBU
        for j in range(5):
            ps = ps_pre(j, NB, "ps_h1")
            for q in range(2):
                nc.tensor.matmul(out=ps[:],
                                 lhsT=wp1p[:, q, :, j * 128:(j + 1) * 128],
                                 rhs=eTs[:, q, :, 0:NB],
                                 start=(q == 0), stop=(q == 1),
                                 perf_mode=mybir.MatmulPerfMode.DoubleRow)
            nc.scalar.activation(h1s[:, j, 0:NB], ps[:], AF.Tanh,
                                 bias=bp1_t[:, j:j + 1], scale=SE8 * SP8)
        for j in range(5):
            ps = ps_pre(j, NB, "ps_p2")
            for q in range(3):
                nc.tensor.matmul(out=ps[:],
                                 lhsT=wp2p[:, q, :, j * 128:(j + 1) * 128],
                                 rhs=h1s[:, 2 * q:2 * q + 2, 0:NB],
                                 start=(q == 0), stop=(q == 2),
                                 perf_mode=mybir.MatmulPerfMode.DoubleRow)
            nc.scalar.activation(pred_s[:, j, 0:NB], ps[:], AF.Tanh,
                                 bias=bp2_t[:, j:j + 1], scale=SP8)
        # ---------------- pp and A interleaved per j: pp's evac chain gates
        # the Pool expansion start, so pp_j runs first, with at_j (enc @
        # Wj1_enc + bj1, transposed layout) filling the PE between evacs
        for j in range(5):
            ps2 = ps_pre(j, GT_TOT, "ps_at")
            for k in range(KJ1_ENC):
                nc.tensor.matmul(out=ps2[:],
                                 lhsT=wj1e_b[k][:, j * 128:(j + 1) * 128],
                                 rhs=encT[k][:, 0:GT_TOT],
                                 start=(k == 0), stop=(k == KJ1_ENC - 1))
            nc.scalar.activation(at_t[j][:], ps2[:], AF.Identity,
                                 bias=bj1_t[:, j:j + 1], scale=1.0)
            ps = ps_pre(j, NB, "ps_pp")
            for q in range(3):
                nc.tensor.matmul(out=ps[:],
                                 lhsT=wj1pp[:, q, :, j * 128:(j + 1) * 128],
                                 rhs=pred_s[:, 2 * q:2 * q + 2, 0:NB],
                                 start=(q == 0), stop=(q == 2),
                                 perf_mode=mybir.MatmulPerfMode.DoubleRow)
            if j >= 3:
                # late-j pp evacs go to ACT: Pool only reads them after its
                # serial add chain reaches j, and keeping them out of DVE's
                # queue lets the first h8/r8 ops start sooner
                nc.scalar.activation(pp_t[j][:], ps[:], AF.Identity,
                                     scale=SP8)
            else:
                nc.vector.tensor_scalar(out=pp_t[j][:], in0=ps[:],
                                        scalar1=SP8, scalar2=None,
                                        op0=mybir.AluOpType.mult)


        # ---------------- ragged expansion: POOL add -> DVE h8, hb, r8
        # each (b, j) split into two g-ranges to halve pipeline latency
        def emit_expand(b, j, g0, gn):
            u1 = UB1[b]
            rv = gn * u1
            off = OFF_R[b] + g0 * u1
            tmp = expand.tile([128, MAXRV], BF16, tag="tmp", name="tmp")
            # first batch, first piece: odd-j adds run on DVE (1.04ns/elt vs
            # Pool-gpsimd 1.98) to halve the serial add chain feeding tile 0
            eng = (nc.vector if (b == BATCH_ORDER[0] and g0 == 0 and j == 1)
                   else nc.gpsimd)
            eng.tensor_tensor(
                out=tmp[:, 0:rv].rearrange("p (g u) -> p g u", g=gn),
                in0=at_t[j][:, OFF_T[b] + g0:OFF_T[b] + g0 + gn][:, :, None]
                    .to_broadcast([128, gn, u1]),
                in1=pp_t[j][:, UOFF[b]:UOFF[b] + u1][:, None, :]
                    .to_broadcast([128, gn, u1]),
                op=mybir.AluOpType.add)
            if b in H8_ACT_BATCHES:
                nc.scalar.activation(hs[:, j, off:off + rv], tmp[:, 0:rv],
                                     AF.Relu, scale=1.0)
            else:
                nc.vector.tensor_scalar(out=hs[:, j, off:off + rv],
                                        in0=tmp[:, 0:rv],
                                        scalar1=0.0, scalar2=None,
                                        op0=mybir.AluOpType.max)
            if j == 4 and b in R4_DROP_BATCHES:
                return  # r8 slot stays zero (memset at start)
            hb = hbpool.tile([128, MAXRV], BF16, tag="hb", name="hb")
            nc.vector.tensor_scalar(out=hb[:, 0:rv], in0=tmp[:, 0:rv],
                                    scalar1=0.0, scalar2=None,
                                    op0=mybir.AluOpType.max)
            nc.vector.tensor_tensor(out=hs[:, 5 + j, off:off + rv],
                                    in0=hb[:, 0:rv], in1=hs[:, j, off:off + rv],
                                    op=mybir.AluOpType.subtract)

        # ---------------- main loop: 64 row-tiles of fp8 DoubleRow matmuls
        def emit_tail_tile(rt, osb, use_main_ps, evac_eng):
            # chunk-major drain overlap for the final tile: finish each
            # 512-col PSUM chunk's pairs (q5 split at SPLIT6 for the stop
            # flag), evacuate and DMA it while the next chunk's matmuls run.
            # Chunks use ps_small buffers (free since the pred net); evacs
            # alternate DVE/ACT so neither queue backs up.
            m = min(128, ROWS - rt * 128)
            for ci, (c0, cn) in enumerate(V_CHUNKS):
                if use_main_ps:
                    psc = ps_main.tile([128, V], F32, tag="ps_out",
                                       name="ps_tail")
                else:
                    psc = ps_small.tile([128, 512], F32, tag="ps_s",
                                        name="ps_tail")
                for q in range(NPAIRS):
                    s = PAIR_HSLOT[q]
                    if SPLIT6 and ci == 0 and q == 5:
                        nc.tensor.matmul(
                            out=psc[0:m, 0:SPLIT6],
                            lhsT=hs[:, s:s + 2, rt * 128: rt * 128 + m],
                            rhs=wp8[q][:, :, 0:SPLIT6],
                            start=False, stop=True,
                            perf_mode=mybir.MatmulPerfMode.DoubleRow)
                        nc.tensor.matmul(
                            out=psc[0:m, SPLIT6:512],
                            lhsT=hs[:, s:s + 2, rt * 128: rt * 128 + m],
                            rhs=wp8[q][:, :, SPLIT6:512],
                            start=False, stop=False,
                            perf_mode=mybir.MatmulPerfMode.DoubleRow)
                        continue
                    if SPLIT6 and ci == 0 and q == 6:
                        nc.tensor.matmul(
                            out=psc[0:m, SPLIT6:512],
                            lhsT=hs[:, s:s + 2, rt * 128: rt * 128 + m],
                            rhs=wp8[q][:, :, SPLIT6:512],
                            start=False, stop=True,
                            perf_mode=mybir.MatmulPerfMode.DoubleRow)
                        continue
                    nc.tensor.matmul(
                        out=psc[0:m, 0:cn],
                        lhsT=hs[:, s:s + 2, rt * 128: rt * 128 + m],
                        rhs=wp8[q][:, :, c0:c0 + cn],
                        start=(q == 0),
                        stop=(q == NPAIRS - 1 and (ci > 0 or not SPLIT6)),
                        perf_mode=mybir.MatmulPerfMode.DoubleRow)
                if evac_eng[ci] == "act":
                    nc.scalar.activation(osb[0:m, c0:c0 + cn],
                                         psc[0:m, 0:cn],
                                         AF.Identity, scale=DESCALE)
                else:
                    nc.vector.tensor_scalar(out=osb[0:m, c0:c0 + cn],
                                            in0=psc[0:m, 0:cn],
                                            scalar1=DESCALE, scalar2=None,
                                            op0=mybir.AluOpType.mult)
                if ci == 2:
                    # one DMA for the whole tile: a single HWDGE slot beats
                    # earlier partial issue on the serialized end-of-stream
                    nc.sync.dma_start(out=out_d[rt * 128: rt * 128 + m, :],
                                      in_=osb[0:m, :])

        def emit_main_tile(rt):
            m = min(128, ROWS - rt * 128)
            ps = ps_main.tile([128, V], F32, tag="ps_out", name="ps_out")
            for q in range(NPAIRS):
                s = PAIR_HSLOT[q]
                chunks = ALL_CH if q < 6 else CH_HI
                for (c0, cn) in chunks:
                    stop = (q == 5 and c0 < SPLIT6) or (q == NPAIRS - 1)
                    nc.tensor.matmul(
                        out=ps[0:m, c0:c0 + cn],
                        lhsT=hs[:, s:s + 2, rt * 128: rt * 128 + m],
                        rhs=wp8[q][:, :, c0:c0 + cn],
                        start=(q == 0), stop=stop,
                        perf_mode=mybir.MatmulPerfMode.DoubleRow)
            osb = outp.tile([128, V], BF16, tag="osb", name="osb")
            nc.scalar.activation(osb[0:m], ps[0:m], AF.Identity,
                                 scale=DESCALE)
            if not bias2_zero:
                nc.vector.tensor_tensor(out=osb[0:m], in0=osb[0:m],
                                        in1=bt_t[0:m],
                                        op=mybir.AluOpType.add)
            nc.sync.dma_start(out=out_d[rt * 128: rt * 128 + m, :],
                              in_=osb[0:m])

        # emit every expansion piece upfront: engine queues are independent,
        # the tmp/hb rings self-pace Pool vs DVE, and hs is persistent, so
        # interleaving with main tiles only adds artificial ordering
        def pieces_for(b):
            g = G[b]
            return [(0, g // 2), (g // 2, g - g // 2)]

        for b in BATCH_ORDER:
            for (g0, gn) in pieces_for(b):
                for j in range(NJ):
                    emit_expand(b, j, g0, gn)
        n_tail = 1 if bias2_zero else 0
        for rt in range(NTILES - n_tail):
            emit_main_tile(rt)
        if bias2_zero:
            osb_t63 = stage.tile([128, V], BF16, tag="osb_t63",
                                 name="osb_t63")
            emit_tail_tile(NTILES - 1, osb_t63, False, ("dve", "act", "dve"))

    nc.compile()
    return nc


def _host_inputs(inputs):
    """Build per-core in_maps from the full inputs."""
    import ml_dtypes
    E4 = ml_dtypes.float8_e4m3
    BF = ml_dtypes.bfloat16

    enc = np.ascontiguousarray(np.asarray(inputs["encoder_states"], dtype=np.float32))
    targets = np.asarray(inputs["targets"]).astype(np.int64)
    emb = np.asarray(inputs["emb"], dtype=np.float32)

    # host-side H-gram context gather: eT[:, b*41+u] = [emb[ext[b,u+1]]; emb[ext[b,u]]]
    ext = np.zeros((B, U + H), np.int64)
    ext[:, H:] = targets
    idx0 = ext[:, 1:U + 2]        # [B, 41]
    idx1 = ext[:, 0:U + 1]
    e0 = emb[idx0] * (idx0 != 0)[..., None]     # [B, 41, 256]
    e1 = emb[idx1] * (idx1 != 0)[..., None]
    eT = np.concatenate([e0, e1], axis=-1)       # [B, 41, 512]
    # pack valid u-slots only (sum(U_b+1) columns, no 41-stride padding)
    eT = np.concatenate([eT[b, :u1] for b, u1 in enumerate(UB1)], axis=0)
    eT = np.ascontiguousarray(eT.T)

    def pack_pairs(w, npair, pad_cols):
        # w [K, M] fp32 -> [npair*128, 2*(M+padding)] fp8 pair-interleaved
        K, M = w.shape
        out = np.zeros((npair, 128, 2, M + pad_cols), np.float32)
        for k in range((K + 127) // 128):
            out[k // 2, :, k % 2, 0:M] = w[k * 128:(k + 1) * 128]
        return out.astype(E4).reshape(npair * 128, 2 * (M + pad_cols))

    # --- W_j2 double-fp8 split (scaled by 1/sw), packed into DoubleRow pairs
    W2s = np.asarray(inputs["W_j2"], dtype=np.float64) / SW
    W8 = W2s.astype(np.float32).astype(E4)
    Rs = (W2s - W8.astype(np.float64)).astype(np.float32)
    R8 = Rs.astype(E4)
    W8k = [W8[k * 128:(k + 1) * 128] for k in range(5)]
    R8k = [R8[k * 128:(k + 1) * 128] for k in range(5)]
    if NPAIRS == 7:
        slots = W8k + [W8k[0]] + R8k[:4]
    else:
        slots = W8k + [W8k[0]] + R8k + [np.zeros((128, V), E4)]
    wsrc = np.concatenate([s.astype(E4) for s in slots], axis=0)

    wj1_s = np.asarray(inputs["W_j1"], dtype=np.float32) / SH
    bj1_s = np.asarray(inputs["b_j1"], dtype=np.float32) / SH

    def per_part(packed, npair):
        # [npair*128, 2*M] -> [128, npair*2*M] per-partition-major
        n2m = packed.shape[1]
        return packed.reshape(npair, 128, n2m).transpose(1, 0, 2).reshape(128, -1)

    wj1e_pp = np.ascontiguousarray(
        wj1_s[:E].astype(BF).reshape(4, 128, J).transpose(1, 0, 2).reshape(128, -1))

    common = {
        "eT": pack_pairs(eT / SE8, 2, NBU_PAD - NBU),
        "w_p1": pack_pairs(np.asarray(inputs["W_pred1"], np.float32) / SP8, 2, 0),
        "w_p2": pack_pairs(np.asarray(inputs["W_pred2"], np.float32) / SP8, 3, 0),
        "w_j1p": pack_pairs(wj1_s[E:] / SP8, 3, 0),
        "w_src": wsrc,
        "b_all": np.ascontiguousarray(np.concatenate([
            np.asarray(inputs["b_pred1"], dtype=np.float32),
            np.asarray(inputs["b_pred2"], dtype=np.float32),
            bj1_s]).reshape(15, 128).T),
        "b_j2": np.asarray(inputs["b_j2"], dtype=np.float32),
    }
    in_maps = []
    for c in range(NCORES):
        enc_sel = np.zeros((GT_PAD, E), np.float32)
        for b in range(B):
            ts = c + NCORES * np.arange(G[b])
            valid = ts < ENC_SIZES[b]
            rows = np.where(valid)[0]
            enc_sel[OFF_T[b] + rows] = enc[b, ts[valid]]
        encT = enc_sel.T.astype(BF)          # [E, GT_PAD]
        encT_pp = encT.reshape(4, 128, GT_PAD).transpose(1, 0, 2).reshape(128, -1)
        pk16 = np.ascontiguousarray(np.concatenate([encT_pp, wj1e_pp], axis=1))
        in_maps.append({"pk16": pk16, **common})
    return in_maps


def _gather_output(core_outs, inputs):
    fb = np.asarray(inputs["flat_b"]).astype(np.int64)
    ft = np.asarray(inputs["flat_t"]).astype(np.int64)
    fu = np.asarray(inputs["flat_u"]).astype(np.int64)
    ub1 = np.asarray(UB1, np.int64)
    off_r = np.asarray([OFF_R[b] for b in range(B)], np.int64)
    core = ft % NCORES
    local = off_r[fb] + (ft // NCORES) * ub1[fb] + fu
    out = np.empty((fb.shape[0], V), np.float32)
    for c in range(NCORES):
        m = core == c
        out[m] = core_outs[c][local[m]].astype(np.float32)
    return out


def kernel(**inputs) -> np.ndarray:
    bias2_zero = not np.any(np.asarray(inputs["b_j2"]))
    key = ("nc", bias2_zero)
    if key not in _cache:
        _cache[key] = _build(bias2_zero=bias2_zero)
        _cache["nc"] = _cache[key]
    nc = _cache[key]
    _cache["nc"] = nc
    in_maps = _host_inputs(inputs)
    for attempt in range(3):
        res = run_bass_kernel_spmd(nc, in_maps, list(range(NCORES))).results
        core_outs = [res[c]["out"] for c in range(NCORES)]
        out = _gather_output(core_outs, inputs)
        # rare transient device flake produces non-finite values; retry
        if np.isfinite(out).all():
            break
    return out



# revision 88
# speedup vs baseline: 1.0013x; 1.0013x over previous
"""RNN-T joiner (nn_CombinationModel_53154515256115) as a Bass/Tile SPMD kernel
for 8 Trainium2 NeuronCores.

Algorithm
---------
For each valid (b, t, u):
    out[b,t,u] = relu(enc[b,t] @ Wj1_enc + pred[b,u] @ Wj1_pred + bj1) @ Wj2 + bj2
The joint pre-activation factors into a per-(b,t) term A and a per-(b,u) term
Pp. The dominant [N,640] @ [640,1056] output matmul runs on the PE in fp8-e4m3
DoubleRow mode (2 fp8 weights per cell, 256-wide contraction per instruction)
with a two-sided residual correction to keep precision:

    h ~= h8 + r8          (h8 = e4m3(h), r8 = e4m3(h - h8))
    W ~= W8 + R8          (host-side split of Wj2, scaled by 1/sw)
    out = h8@W8 + r8@W8 + h8@R8      (r8@R8 ~ 1e-3 relative, dropped)

All three block-products are packed into 7 DoubleRow k-pairs over a 10-slot
fp8 "h stack" (slots 0-4 = h8 k-tiles, 5-9 = r8 k-tiles); the W-side pair
tiles are prebuilt on the host so each pair picks the right (W8|R8) slices.
W k-tile 4 stays uncorrected (the 8th pair is dropped), and the r8 residual
for k-tile 4 is additionally skipped on the early small batches
(R4_DROP_BATCHES) to relieve the DVE/Pool expansion pipeline during the
fill phase; measured rel_rms 1.43e-2 vs the 2e-2 gate. Scales sh=1/16
(folded into Wj1 on the host) and sw=1/512 keep everything in e4m3 normal
range; the combined 2^-13 descale is folded into the PSUM evacuation.
Output is written bf16 and upcast on the host.

Schedule highlights (cost-model-driven; 195.8us -> 120.9us):
  - serial-DMA-device order: ball(p-major), eT, wp1, wp2, pk16, wj1p,
    wsrc W-slots (x2), wsrc R-slots (deferred; first read at pair q5)
  - pred-net/at PSUM tiles alternate ps_small/ps_main (4 effective buffers
    halve the PE<->ACT ping-pong latency); at_j and pp_j interleave per j
    so pp's evac chain (which gates the Pool expansion start) begins ASAP
  - expansion: Pool broadcast-add, DVE h8/hb/r8; all pieces emitted
    upfront (rings self-pace); pred-net u-axis packed to 296 valid slots
  - final tile runs chunk-major (per-512-col PSUM chunks evacuated on
    DVE/ACT and DMA'd while later chunks compute) to cut the drain tail

Sharding (SPMD-uniform)
-----------------------
Core c takes encoder frames t with t % 8 == c from every batch. Batches are
laid out smallest-first so the first output row-tiles become ready with the
least expansion work. The tiny prediction network (296 rows) is computed
replicated on every core.
"""

import numpy as np

import concourse.bass as bass
import concourse.mybir as mybir
import concourse.tile as tile
from concourse import bacc
from concourse.masks import make_identity
from concourse.bass import IndirectOffsetOnAxis
from concourse.bass_utils import run_bass_kernel_spmd

F32 = mybir.dt.float32
BF16 = mybir.dt.bfloat16
FP8 = mybir.dt.float8e4
I32 = mybir.dt.int32
AF = mybir.ActivationFunctionType

# ---------------------------------------------------------------- constants
B, T, U = 8, 300, 40
E, P, J, V = 512, 640, 640, 1056
H, DEMB = 2, 256
ENC_SIZES = [300, 280, 260, 240, 220, 210, 205, 200]
TGT_SIZES = [40, 38, 35, 33, 30, 28, 26, 25]
NCORES = 8
N_FLAT = 64385

G = [(t + NCORES - 1) // NCORES for t in ENC_SIZES]       # groups/core/batch
UB1 = [u + 1 for u in TGT_SIZES]                          # u-extent per batch
RBV = [G[b] * UB1[b] for b in range(B)]                   # valid rows/batch
ROWS = sum(RBV)                                           # 8134 rows/core
GT_TOT = sum(G)                                           # 242 enc frames/core
GT_PAD = 256
OFF_T = [0]
for b in range(B):
    OFF_T.append(OFF_T[-1] + G[b])

# batches laid out smallest-first in the row dimension
BATCH_ORDER = sorted(range(B), key=lambda b: RBV[b])
OFF_R = {}
_acc = 0
for b in BATCH_ORDER:
    OFF_R[b] = _acc
    _acc += RBV[b]

UOFF = [0]                    # packed (b,u) offsets: sum(U_b+1) = 296
for b in range(B):
    UOFF.append(UOFF[-1] + UB1[b])
NBU = UOFF[-1]                # 296 valid u-slots (vs 41*B = 328 padded)
NBU_PAD = 304                 # eT column pad (% 16)
KJ1_ENC = E // 128            # 4 k-tiles of W_j1 enc part
NJ = J // 128                 # 5 partition tiles of the 640-dim feature axis
V_CHUNKS = [(0, 512), (512, 512), (1024, V - 1024)]
MAXRV = max(RBV)

SH = 1.0 / 16.0               # h scale  (folded into Wj1/bj1 on host)
SP8 = 1.0 / 64.0              # pred-net fp8 weight scale (undone at activations)
SE8 = 1.0 / 64.0              # embedding fp8 scale
SW = 1.0 / 512.0              # W2 scale (folded into W8/R8 on host)
SEE8 = 1.0 / 16.0             # enc fp8 scale (at-loop DoubleRow)
SWE8 = 1.0 / 128.0            # wj1e fp8 scale (at-loop DoubleRow)
DESCALE = SH * SW             # 2^-13, exact

ROWS_PAD = 8192               # slot stride in the h-stack; % 16 == 0
NSLOT = 10                    # 5 h8 + 5 r8
# DoubleRow pair table: (h-slot pair base, W-source description)
# pairs q: h-side slots (2q mod 10, +1); W tiles prebuilt host-side:
#   q0 (h0,h1)x(W0,W1)  q1 (h2,h3)x(W2,W3)  q2 (h4,r0)x(W4,W0)
#   q3 (r1,r2)x(W1,W2)  q4 (r3,r4)x(W3,W4)
#   q5 (h0,h1)x(R0,R1)  q6 (h2,h3)x(R2,R3)  q7 (h4,r0)x(R4,0)
# NPAIRS=7 drops the (h8_4, r8_0) x (R8_4, 0) pair: leaves W k-tile 4
# uncorrected on the W side (~1.2e-2 predicted rel err vs 3.0e-3 for 8)
NPAIRS = 7
PAIR_HSLOT = [0, 2, 4, 6, 8, 0, 2, 4][:NPAIRS]
# emission order within a tile: h8-only pairs first so early tiles can
# start before the (later-ready) r8 slots exist
PAIR_ORDER = [0, 5, 1, 6, 2, 3, 4]
WSRC_OFF = (0, 2, 4, 1, 3, 6, 8, 10)[:NPAIRS]
NWSLOT = 10 if NPAIRS == 7 else 12

# columns [0, SPLIT6) also skip the q6 (h8 x R2,R3) correction pair: those
# columns carry W-quant error on 3/5 k-tiles (2.15e-2) instead of 1/5
# (1.24e-2); blended rel_rms ~= sqrt(f*4.62 + (1-f)*1.54)e-2 ~= 1.6e-2 at
# f = 1/3, saving SPLIT6/2 PE cycles per row-tile
SPLIT6 = 0
CH_LO = [(0, SPLIT6)] if SPLIT6 else []
CH_HI = ([(SPLIT6, 512 - SPLIT6)] if SPLIT6 else []) + \
    [(512, 512), (1024, V - 1024)] + ([] if SPLIT6 else [(0, 512)])
CH_HI.sort()
ALL_CH = sorted(CH_LO + CH_HI)

NTILES = (ROWS + 127) // 128
# batches whose h8-relu runs on ACT instead of DVE (engine balancing);
# empty: ACT must stay dedicated to PSUM evacuation or the PE stalls
H8_ACT_BATCHES = set()
# r8 (h-residual) is skipped for j=4 on the early batches: spends error
# budget (h-quant on 1/5 of K for ~37%% of rows, +~0.8e-2 rms in quadrature)
# to cut DVE expansion work exactly where the fill-phase lag stalls the PE.
# ACT can't help instead: its in-order queue delays PSUM evac (-> 144us).
R4_DROP_BATCHES = {7, 6, 5, 4}
FILL_AT = 0                  # PE fillers before the at-loop (pk16 wait)
FILL_MAIN = 0                 # PE fillers before main tile 0 (hs wait)

_cache = {}


def _build(bias2_zero=True):
    nc = bacc.Bacc("TRN2", target_bir_lowering=False, debug=False,
                   num_devices=NCORES)

    eT_d = nc.dram_tensor("eT", [2 * 128, 2 * NBU_PAD], FP8, kind="ExternalInput").ap()
    wp1_d = nc.dram_tensor("w_p1", [2 * 128, 2 * P], FP8, kind="ExternalInput").ap()
    wp2_d = nc.dram_tensor("w_p2", [3 * 128, 2 * P], FP8, kind="ExternalInput").ap()
    wj1p_d = nc.dram_tensor("w_j1p", [3 * 128, 2 * J], FP8, kind="ExternalInput").ap()
    # packed bf16: [encT | wj1e]
    PK16 = 4 * GT_PAD + 4 * J
    pk16_d = nc.dram_tensor("pk16", [128, PK16], BF16, kind="ExternalInput").ap()
    wsrc_d = nc.dram_tensor("w_src", [NWSLOT * 128, V], FP8,
                             kind="ExternalInput").ap()
    ball_d = nc.dram_tensor("b_all", [128, 15], F32, kind="ExternalInput").ap()
    bj2_d = nc.dram_tensor("b_j2", [V], F32, kind="ExternalInput").ap()
    out_d = nc.dram_tensor("out", [ROWS, V], BF16, kind="ExternalOutput").ap()

    from contextlib import ExitStack
    with tile.TileContext(nc) as tc, ExitStack() as ctx:
        persist = ctx.enter_context(tc.tile_pool(name="persist", bufs=1))
        stage = ctx.enter_context(tc.tile_pool(name="stage", bufs=1))
        expand = ctx.enter_context(tc.tile_pool(name="expand", bufs=5))
        hbpool = ctx.enter_context(tc.tile_pool(name="hbpool", bufs=3))
        outp = ctx.enter_context(tc.tile_pool(name="outp", bufs=3))
        ps_small = ctx.enter_context(tc.tile_pool(name="ps_small", bufs=2, space="PSUM"))
        ps_main = ctx.enter_context(tc.tile_pool(name="ps_main", bufs=2, space="PSUM"))

        # ---------------- PE warmup: build a >3us busy streak while DMAs run
        warm = stage.tile([128, 256], BF16, tag="warm", name="warm")
        warm_ps = [None]

        def emit_fillers(n):
            # dummy matmuls keep the PE busy-streak alive across known DMA
            # waits so the preamble runs at full p-state (idle >100ns drops
            # the clock to 1.2GHz for the next 3us of work)
            for _ in range(n):
                # main ring, not ps_small: keeps the pred-net's PSUM slots
                # free of WAW deps against the warm stream
                psw = ps_main.tile([128, V], F32, tag="ps_out",
                                   name="ps_warm")
                nc.tensor.matmul(out=psw[0:128, 0:256], lhsT=warm[:, 0:128],
                                 rhs=warm[:], start=True, stop=True)

        nc.vector.memset(warm[:], 0.0)
        # dummy gpsimd op hoists the Q7 library load into the idle window
        glib = stage.tile([128, 16], BF16, tag="glib", name="glib")
        nc.gpsimd.memset(glib[:], 0.0)
        emit_fillers(14)

        # ---------------- persistent SBUF state
        hs = persist.tile([128, NSLOT, ROWS_PAD], FP8, tag="hs", name="hs")
        drop_rows = sum(RBV[b] for b in R4_DROP_BATCHES)
        if drop_rows:
            # dropped r4 region must read as zero in pair q4 (DVE is idle
            # for ~9us at the start, so this costs no critical-path time)
            nc.vector.memset(hs[:, 9, 0:drop_rows], 0.0)
        at_t = [persist.tile([128, GT_TOT], F32, tag=f"at_{j}", name=f"at_{j}")
                for j in range(NJ)]
        pp_t = [persist.tile([128, NBU], F32, tag=f"pp_{j}", name=f"pp_{j}")
                for j in range(NJ)]

        # ---------------- weight loads: one strided DMA per tensor, ordered
        # by when the dependent compute needs them (DMA device is serial).
        # wsrc splits into the W-slots (needed at main-loop start) and the
        # R-slots (first read by pair q5, much later). ball ships [128,15]
        # p-major (contiguous 60B/partition descriptors) and goes first: the
        # biases gate every activation stage.
        ball_t = stage.tile([128, 15], F32, tag="ball")
        nc.sync.dma_start(out=ball_t[:], in_=ball_d)
        bp1_t = ball_t[:, 0:5]
        bp2_t = ball_t[:, 5:10]
        bj1_t = ball_t[:, 10:15]
        wp1p = stage.tile([128, 2, 2, P], FP8, tag="wp1", name="wp1")
        nc.sync.dma_start(out=wp1p[:].rearrange("p q a b -> p q (a b)"),
                          in_=wp1_d.rearrange("(q p) x -> p q x", p=128))
        eTs = stage.tile([128, 2, 2, NBU_PAD], FP8, tag="eT", name="eT")
        nc.sync.dma_start(out=eTs[:].rearrange("p q a b -> p q (a b)"),
                          in_=eT_d.rearrange("(q p) x -> p q x", p=128))
        wp2p = stage.tile([128, 3, 2, P], FP8, tag="wp2", name="wp2")
        nc.sync.dma_start(out=wp2p[:].rearrange("p q a b -> p q (a b)"),
                          in_=wp2_d.rearrange("(q p) x -> p q x", p=128))
        pk16 = stage.tile([128, PK16], BF16, tag="pk16", name="pk16")
        nc.sync.dma_start(out=pk16[:], in_=pk16_d)
        encT = [pk16[:, f * GT_PAD:(f + 1) * GT_PAD] for f in range(KJ1_ENC)]
        wj1e_b = [pk16[:, 4 * GT_PAD + k * J:4 * GT_PAD + (k + 1) * J]
                  for k in range(4)]
        wj1pp = stage.tile([128, 3, 2, J], FP8, tag="wj1p", name="wj1p")
        nc.sync.dma_start(out=wj1pp[:].rearrange("p q a b -> p q (a b)"),
                          in_=wj1p_d.rearrange("(q p) x -> p q x", p=128))
        if not bias2_zero:
            bt_t = persist.tile([128, V], BF16, tag="bt", name="bt")
            nc.sync.dma_start(out=bt_t[:], in_=bj2_d[None, :].to_broadcast([128, V]))
        # 12 W-slots [W0,W1,W2,W3,W4,W0, R0,R1,R2,R3,R4, 0]; each DoubleRow
        # pair reads two adjacent slots (overlapping views give the shifted
        # (W1,W2)/(W3,W4) pairs without duplicate storage)
        wsrc = persist.tile([128, NWSLOT, V], FP8, tag="wsrc", name="wsrc")
        nc.sync.dma_start(out=wsrc[:],
                          in_=wsrc_d.rearrange("(s p) v -> p s v", p=128))
        wp8 = [wsrc[:, o:o + 2] for o in WSRC_OFF]

        # ---------------- prediction network (replicated, 328 rows, fp8 pairs)
        # PSUM tiles alternate between ps_small and the main ring (idle until
        # tile 0) — 4 effective buffers halve the PE<->ACT ping-pong latency
        # that otherwise delays the expansion start by ~4us
        def ps_pre(j, n, name):
            if j % 2:
                return ps_main.tile([128, V], F32, tag="ps_out",
                                    name=name)[:, 0:n]
            return ps_small.tile([128, n], F32, tag="ps_s", name=name)

        h1s = stage.tile([128, 6, NBU_PAD], FP8, tag="h1s", name="h1s")
        pred_s = stage.tile([128, 6, NBU_PAD], FP8, tag="pred_s", name="pred_s")
        nc.vector.memset(h1s[:, 5], 0.0)
        nc.vector.memset(pred_s[:, 5], 0.0)
        NB = NBU
        for j in range(5):
            ps = ps_pre(j, NB, "ps_h1")
            for q in range(2):
                nc.tensor.matmul(out=ps[:],
                                 lhsT=wp1p[:, q, :, j * 128:(j + 1) * 128],
                                 rhs=eTs[:, q, :, 0:NB],
                                 start=(q == 0), stop=(q == 1),
                                 perf_mode=mybir.MatmulPerfMode.DoubleRow)
            nc.scalar.activation(h1s[:, j, 0:NB], ps[:], AF.Tanh,
                                 bias=bp1_t[:, j:j + 1], scale=SE8 * SP8)
        for j in range(5):
            ps = ps_pre(j, NB, "ps_p2")
            for q in range(3):
                nc.tensor.matmul(out=ps[:],
                                 lhsT=wp2p[:, q, :, j * 128:(j + 1) * 128],
                                 rhs=h1s[:, 2 * q:2 * q + 2, 0:NB],
                                 start=(q == 0), stop=(q == 2),
                                 perf_mode=mybir.MatmulPerfMode.DoubleRow)
            nc.scalar.activation(pred_s[:, j, 0:NB], ps[:], AF.Tanh,
                                 bias=bp2_t[:, j:j + 1], scale=SP8)
        # ---------------- pp and A interleaved per j: pp's evac chain gates
        # the Pool expansion start, so pp_j runs first, with at_j (enc @
        # Wj1_enc + bj1, transposed layout) filling the PE between evacs
        for j in range(5):
            ps2 = ps_pre(j, GT_TOT, "ps_at")
            for k in range(KJ1_ENC):
                nc.tensor.matmul(out=ps2[:],
                                 lhsT=wj1e_b[k][:, j * 128:(j + 1) * 128],
                                 rhs=encT[k][:, 0:GT_TOT],
                                 start=(k == 0), stop=(k == KJ1_ENC - 1))
            nc.scalar.activation(at_t[j][:], ps2[:], AF.Identity,
                                 bias=bj1_t[:, j:j + 1], scale=1.0)
            ps = ps_pre(j, NB, "ps_pp")
            for q in range(3):
                nc.tensor.matmul(out=ps[:],
                                 lhsT=wj1pp[:, q, :, j * 128:(j + 1) * 128],
                                 rhs=pred_s[:, 2 * q:2 * q + 2, 0:NB],
                                 start=(q == 0), stop=(q == 2),
                                 perf_mode=mybir.MatmulPerfMode.DoubleRow)
            if j >= 3:
                # late-j pp evacs go to ACT: Pool only reads them after its
                # serial add chain reaches j, and keeping them out of DVE's
                # queue lets the first h8/r8 ops start sooner
                nc.scalar.activation(pp_t[j][:], ps[:], AF.Identity,
                                     scale=SP8)
            else:
                nc.vector.tensor_scalar(out=pp_t[j][:], in0=ps[:],
                                        scalar1=SP8, scalar2=None,
                                        op0=mybir.AluOpType.mult)


        # ---------------- ragged expansion: POOL add -> DVE h8, hb, r8
        # each (b, j) split into two g-ranges to halve pipeline latency
        def emit_expand(b, j, g0, gn):
            u1 = UB1[b]
            rv = gn * u1
            off = OFF_R[b] + g0 * u1
            tmp = expand.tile([128, MAXRV], BF16, tag="tmp", name="tmp")
            # first batch, first piece: odd-j adds run on DVE (1.04ns/elt vs
            # Pool-gpsimd 1.98) to halve the serial add chain feeding tile 0
            eng = (nc.vector if (b == BATCH_ORDER[0] and g0 == 0 and j == 1)
                   else nc.gpsimd)
            eng.tensor_tensor(
                out=tmp[:, 0:rv].rearrange("p (g u) -> p g u", g=gn),
                in0=at_t[j][:, OFF_T[b] + g0:OFF_T[b] + g0 + gn][:, :, None]
                    .to_broadcast([128, gn, u1]),
                in1=pp_t[j][:, UOFF[b]:UOFF[b] + u1][:, None, :]
                    .to_broadcast([128, gn, u1]),
                op=mybir.AluOpType.add)
            if b in H8_ACT_BATCHES:
                nc.scalar.activation(hs[:, j, off:off + rv], tmp[:, 0:rv],
                                     AF.Relu, scale=1.0)
            else:
                nc.vector.tensor_scalar(out=hs[:, j, off:off + rv],
                                        in0=tmp[:, 0:rv],
                                        scalar1=0.0, scalar2=None,
                                        op0=mybir.AluOpType.max)
            if j == 4 and b in R4_DROP_BATCHES:
                return  # r8 slot stays zero (memset at start)
            hb = hbpool.tile([128, MAXRV], BF16, tag="hb", name="hb")
            nc.vector.tensor_scalar(out=hb[:, 0:rv], in0=tmp[:, 0:rv],
                                    scalar1=0.0, scalar2=None,
                                    op0=mybir.AluOpType.max)
            nc.vector.tensor_tensor(out=hs[:, 5 + j, off:off + rv],
                                    in0=hb[:, 0:rv], in1=hs[:, j, off:off + rv],
                                    op=mybir.AluOpType.subtract)

        # ---------------- main loop: 64 row-tiles of fp8 DoubleRow matmuls
        def emit_tail_tile(rt, osb, use_main_ps, evac_eng):
            # chunk-major drain overlap for the final tile: finish each
            # 512-col PSUM chunk's pairs (q5 split at SPLIT6 for the stop
            # flag), evacuate and DMA it while the next chunk's matmuls run.
            # Chunks use ps_small buffers (free since the pred net); evacs
            # alternate DVE/ACT so neither queue backs up.
            m = min(128, ROWS - rt * 128)
            for ci, (c0, cn) in enumerate(V_CHUNKS):
                if use_main_ps:
                    psc = ps_main.tile([128, V], F32, tag="ps_out",
                                       name="ps_tail")
                else:
                    psc = ps_small.tile([128, 512], F32, tag="ps_s",
                                        name="ps_tail")
                for q in range(NPAIRS):
                    s = PAIR_HSLOT[q]
                    if SPLIT6 and ci == 0 and q == 5:
                        nc.tensor.matmul(
                            out=psc[0:m, 0:SPLIT6],
                            lhsT=hs[:, s:s + 2, rt * 128: rt * 128 + m],
                            rhs=wp8[q][:, :, 0:SPLIT6],
                            start=False, stop=True,
                            perf_mode=mybir.MatmulPerfMode.DoubleRow)
                        nc.tensor.matmul(
                            out=psc[0:m, SPLIT6:512],
                            lhsT=hs[:, s:s + 2, rt * 128: rt * 128 + m],
                            rhs=wp8[q][:, :, SPLIT6:512],
                            start=False, stop=False,
                            perf_mode=mybir.MatmulPerfMode.DoubleRow)
                        continue
                    if SPLIT6 and ci == 0 and q == 6:
                        nc.tensor.matmul(
                            out=psc[0:m, SPLIT6:512],
                            lhsT=hs[:, s:s + 2, rt * 128: rt * 128 + m],
                            rhs=wp8[q][:, :, SPLIT6:512],
                            start=False, stop=True,
                            perf_mode=mybir.MatmulPerfMode.DoubleRow)
                        continue
                    nc.tensor.matmul(
                        out=psc[0:m, 0:cn],
                        lhsT=hs[:, s:s + 2, rt * 128: rt * 128 + m],
                        rhs=wp8[q][:, :, c0:c0 + cn],
                        start=(q == 0),
                        stop=(q == NPAIRS - 1 and (ci > 0 or not SPLIT6)),
                        perf_mode=mybir.MatmulPerfMode.DoubleRow)
                if evac_eng[ci] == "act":
                    nc.scalar.activation(osb[0:m, c0:c0 + cn],
                                         psc[0:m, 0:cn],
                                         AF.Identity, scale=DESCALE)
                else:
                    nc.vector.tensor_scalar(out=osb[0:m, c0:c0 + cn],
                                            in0=psc[0:m, 0:cn],
                                            scalar1=DESCALE, scalar2=None,
                                            op0=mybir.AluOpType.mult)
                if ci == 2:
                    # one DMA for the whole tile: a single HWDGE slot beats
                    # earlier partial issue on the serialized end-of-stream
                    nc.sync.dma_start(out=out_d[rt * 128: rt * 128 + m, :],
                                      in_=osb[0:m, :])

        def emit_main_tile(rt):
            m = min(128, ROWS - rt * 128)
            ps = ps_main.tile([128, V], F32, tag="ps_out", name="ps_out")
            for qi, q in enumerate(PAIR_ORDER):
                s = PAIR_HSLOT[q]
                for (c0, cn) in ALL_CH:
                    nc.tensor.matmul(
                        out=ps[0:m, c0:c0 + cn],
                        lhsT=hs[:, s:s + 2, rt * 128: rt * 128 + m],
                        rhs=wp8[q][:, :, c0:c0 + cn],
                        start=(qi == 0), stop=(qi == NPAIRS - 1),
                        perf_mode=mybir.MatmulPerfMode.DoubleRow)
            osb = outp.tile([128, V], BF16, tag="osb", name="osb")
            nc.scalar.activation(osb[0:m], ps[0:m], AF.Identity,
                                 scale=DESCALE)
            if not bias2_zero:
                nc.vector.tensor_tensor(out=osb[0:m], in0=osb[0:m],
                                        in1=bt_t[0:m],
                                        op=mybir.AluOpType.add)
            nc.sync.dma_start(out=out_d[rt * 128: rt * 128 + m, :],
                              in_=osb[0:m])

        # emit every expansion piece upfront: engine queues are independent,
        # the tmp/hb rings self-pace Pool vs DVE, and hs is persistent, so
        # interleaving with main tiles only adds artificial ordering
        def pieces_for(b):
            g = G[b]
            return [(0, g // 2), (g // 2, g - g // 2)]

        for b in BATCH_ORDER:
            for (g0, gn) in pieces_for(b):
                for j in range(NJ):
                    emit_expand(b, j, g0, gn)
        n_tail = 1 if bias2_zero else 0
        for rt in range(NTILES - n_tail):
            emit_main_tile(rt)
        if bias2_zero:
            osb_t63 = stage.tile([128, V], BF16, tag="osb_t63",
                                 name="osb_t63")
            emit_tail_tile(NTILES - 1, osb_t63, False, ("dve", "act", "dve"))

    nc.compile()
    return nc


def _host_inputs(inputs):
    """Build per-core in_maps from the full inputs."""
    import ml_dtypes
    E4 = ml_dtypes.float8_e4m3
    BF = ml_dtypes.bfloat16

    enc = np.ascontiguousarray(np.asarray(inputs["encoder_states"], dtype=np.float32))
    targets = np.asarray(inputs["targets"]).astype(np.int64)
    emb = np.asarray(inputs["emb"], dtype=np.float32)

    # host-side H-gram context gather: eT[:, b*41+u] = [emb[ext[b,u+1]]; emb[ext[b,u]]]
    ext = np.zeros((B, U + H), np.int64)
    ext[:, H:] = targets
    idx0 = ext[:, 1:U + 2]        # [B, 41]
    idx1 = ext[:, 0:U + 1]
    e0 = emb[idx0] * (idx0 != 0)[..., None]     # [B, 41, 256]
    e1 = emb[idx1] * (idx1 != 0)[..., None]
    eT = np.concatenate([e0, e1], axis=-1)       # [B, 41, 512]
    # pack valid u-slots only (sum(U_b+1) columns, no 41-stride padding)
    eT = np.concatenate([eT[b, :u1] for b, u1 in enumerate(UB1)], axis=0)
    eT = np.ascontiguousarray(eT.T)

    def pack_pairs(w, npair, pad_cols):
        # w [K, M] fp32 -> [npair*128, 2*(M+padding)] fp8 pair-interleaved
        K, M = w.shape
        out = np.zeros((npair, 128, 2, M + pad_cols), np.float32)
        for k in range((K + 127) // 128):
            out[k // 2, :, k % 2, 0:M] = w[k * 128:(k + 1) * 128]
        return out.astype(E4).reshape(npair * 128, 2 * (M + pad_cols))

    # --- W_j2 double-fp8 split (scaled by 1/sw), packed into DoubleRow pairs
    W2s = np.asarray(inputs["W_j2"], dtype=np.float64) / SW
    W8 = W2s.astype(np.float32).astype(E4)
    Rs = (W2s - W8.astype(np.float64)).astype(np.float32)
    R8 = Rs.astype(E4)
    W8k = [W8[k * 128:(k + 1) * 128] for k in range(5)]
    R8k = [R8[k * 128:(k + 1) * 128] for k in range(5)]
    if NPAIRS == 7:
        slots = W8k + [W8k[0]] + R8k[:4]
    else:
        slots = W8k + [W8k[0]] + R8k + [np.zeros((128, V), E4)]
    wsrc = np.concatenate([s.astype(E4) for s in slots], axis=0)

    wj1_s = np.asarray(inputs["W_j1"], dtype=np.float32) / SH
    bj1_s = np.asarray(inputs["b_j1"], dtype=np.float32) / SH

    def per_part(packed, npair):
        # [npair*128, 2*M] -> [128, npair*2*M] per-partition-major
        n2m = packed.shape[1]
        return packed.reshape(npair, 128, n2m).transpose(1, 0, 2).reshape(128, -1)

    wj1e_pp = np.ascontiguousarray(
        wj1_s[:E].astype(BF).reshape(4, 128, J).transpose(1, 0, 2).reshape(128, -1))

    common = {
        "eT": pack_pairs(eT / SE8, 2, NBU_PAD - NBU),
        "w_p1": pack_pairs(np.asarray(inputs["W_pred1"], np.float32) / SP8, 2, 0),
        "w_p2": pack_pairs(np.asarray(inputs["W_pred2"], np.float32) / SP8, 3, 0),
        "w_j1p": pack_pairs(wj1_s[E:] / SP8, 3, 0),
        "w_src": wsrc,
        "b_all": np.ascontiguousarray(np.concatenate([
            np.asarray(inputs["b_pred1"], dtype=np.float32),
            np.asarray(inputs["b_pred2"], dtype=np.float32),
            bj1_s]).reshape(15, 128).T),
        "b_j2": np.asarray(inputs["b_j2"], dtype=np.float32),
    }
    in_maps = []
    for c in range(NCORES):
        enc_sel = np.zeros((GT_PAD, E), np.float32)
        for b in range(B):
            ts = c + NCORES * np.arange(G[b])
            valid = ts < ENC_SIZES[b]
            rows = np.where(valid)[0]
            enc_sel[OFF_T[b] + rows] = enc[b, ts[valid]]
        encT = enc_sel.T.astype(BF)          # [E, GT_PAD]
        encT_pp = encT.reshape(4, 128, GT_PAD).transpose(1, 0, 2).reshape(128, -1)
        pk16 = np.ascontiguousarray(np.concatenate([encT_pp, wj1e_pp], axis=1))
        in_maps.append({"pk16": pk16, **common})
    return in_maps


def _gather_output(core_outs, inputs):
    fb = np.asarray(inputs["flat_b"]).astype(np.int64)
    ft = np.asarray(inputs["flat_t"]).astype(np.int64)
    fu = np.asarray(inputs["flat_u"]).astype(np.int64)
    ub1 = np.asarray(UB1, np.int64)
    off_r = np.asarray([OFF_R[b] for b in range(B)], np.int64)
    core = ft % NCORES
    local = off_r[fb] + (ft // NCORES) * ub1[fb] + fu
    out = np.empty((fb.shape[0], V), np.float32)
    for c in range(NCORES):
        m = core == c
        out[m] = core_outs[c][local[m]].astype(np.float32)
    return out


def kernel(**inputs) -> np.ndarray:
    bias2_zero = not np.any(np.asarray(inputs["b_j2"]))
    key = ("nc", bias2_zero)
    if key not in _cache:
        _cache[key] = _build(bias2_zero=bias2_zero)
        _cache["nc"] = _cache[key]
    nc = _cache[key]
    _cache["nc"] = nc
    in_maps = _host_inputs(inputs)
    for attempt in range(3):
        res = run_bass_kernel_spmd(nc, in_maps, list(range(NCORES))).results
        core_outs = [res[c]["out"] for c in range(NCORES)]
        out = _gather_output(core_outs, inputs)
        # rare transient device flake produces non-finite values; retry
        if np.isfinite(out).all():
            break
    return out

